# revision 12
# baseline (speedup 1.0000x reference)
"""Trainium2 Bass kernel for the CAM-drop attention module.

Reference computation (per sample n):
    cams  = relu(W @ x[n])            # W: [C=64, Cin=1024], x[n]: [Cin, H*W]
    thr_k = gama * max_hw(cams[k])    # per-channel spatial max
    drop  = where(cams > thr, 0, cams)
    out[n] = x[n] * mean_k(drop)      # broadcast over Cin

Data-parallel over the batch: 32 samples sharded 4-per-core across 8
NeuronCores; fc_weights / gama replicated. No cross-core communication.

The problem is HBM-bound (~420 GB/s/core observed; 25.7 MB in + 25.7 MB
out as bf16 -> ~122.5us of pure DMA), so x is pre-cast to bf16 on the
host, the output is stored as bf16 and widened on the host, and the whole
kernel is organized so the serial DVE stream (the per-sample stats +
products chain) finishes before the store stream drains.

Structure (v7):
  - spatial split into 8 chunks of 392; cams for chunk pair (2b, 2b+1)
    packed on PSUM bank b as partitions [0:64) / [64:128) via the PE
    tile_position feature (out partition base 64 for the odd chunk), so
    the per-channel stats ops run on all 128 DVE lanes (~half the cost of
    the 64-partition layout) and cams needs only banks 0-3
  - sample 0 streams t-outer (chasing the initial loads); samples 1+ go
    bank-pair serial so only 2 banks are live per group and sample n+1's
    matmuls never wait on sample n's mean work (banks are disjoint)
  - per-channel spatial max: two partial reduces + final on [128] lanes;
    the cross-partition-half max (channel c lives on lanes c and c+64) is
    resolved by two tiny SBUF->SBUF DMAs on the otherwise-idle scalar
    ring, then thr = gama * max on DVE
  - drop mask via in-place scalar_tensor_tensor (is_le + mult) on [128]
    lanes, two ops covering chunk halves so the mean matmuls chase
  - channel mean via ones-matmuls into ONE 4-bank PSUM tile, two waves
    (chunks 0-3 then 4-7, selected by zero-padded onesA/onesB lhsT that
    pick the partition half), each evacuated by a single strided ACT
    copy, so the first product piece starts ~2us after the masks
  - products in place on the xb tiles (DVE tensor_tensor, 2x_1P); tile 0
    in two halves chasing the mean waves; stores per tile
  - DMA queue discipline: all loads on the sync HWDGE ring; bulk stores
    deferred behind a SWDGE "gate" transfer that reads load tile 27, so
    loads keep the full ~420 GB/s until nearly done (stores otherwise
    halve the load rate from ~37us and push the last sample's entire
    dependent chain out by ~25us); sample 0 tiles 0-1 store immediately
    (their SBUF slots are recycled by loads 30-31); the last sample's
    stores go on the scalar HWDGE ring (~0.6us completion vs ~2us SWDGE)

Measured pitfalls baked into the structure: GpSimd tensor ops contend
with DVE tensor_tensor for the shared SBUF read port (both ~4x slower);
ScalarE ACTIVATE has no 16-bit accel; HAM power throttling (50%-duty
windows ~60% of the time) stretches PE matmuls ~1.65x and everything
else ~5-10%, so all cross-engine chases need slack.
"""

import numpy as np

# Problem shape (hardcoded per harness contract).
N, CIN, H, W = 32, 1024, 56, 56
C = 64
HW = H * W          # 3136
NCORES = 8
NS = N // NCORES    # 4 samples per core
P = 128             # SBUF partitions
NT = CIN // P       # 8 Cin tiles
NCH = 8             # spatial chunks per sample (pairs packed per bank)
CH = HW // NCH      # 392
NB = NCH // 2       # 4 cams PSUM banks
BANK = 512          # PSUM bank stride in f32 elements
NBBUF = 30          # rotating bf16 x-tile slots (0.784 MB each)

_CACHE = {}


def _build_nc():
    from concourse import bacc, bass, tile
    from concourse import mybir

    f32 = mybir.dt.float32
    bf16 = mybir.dt.bfloat16
    alu = mybir.AluOpType

    nc = bacc.Bacc("TRN2", target_bir_lowering=False, debug=False)
    x_ext = nc.declare_dram_parameter("x", [NS, CIN, HW], bf16, isOutput=False)
    # fc_weights prelaid on host as [p, t*C+c] = w[c, t*128+p]: contiguous
    # 1KB partition lines -> one efficient DMA.
    w_ext = nc.declare_dram_parameter("fc_weights", [P, NT * C], bf16, isOutput=False)
    g_ext = nc.declare_dram_parameter("gama", [P, 2], f32, isOutput=False)
    out_ext = nc.declare_dram_parameter("out", [NS, CIN, HW], bf16, isOutput=True)

    with tile.TileContext(nc) as tc:
        with (
            tc.tile_pool(name="consts", bufs=1) as constp,
            tc.tile_pool(name="xbp", bufs=NBBUF) as xbp,
            tc.tile_pool(name="stats", bufs=2) as statp,
            tc.tile_pool(name="camsb", bufs=1) as camp,
            tc.tile_pool(name="meanp", bufs=1) as meanp,
            tc.tile_pool(name="gatep", bufs=1) as gatep,
            tc.tile_pool(name="psum", bufs=1, space=bass.MemorySpace.PSUM) as psump,
        ):
            all_xbs = []
            deferred = []

            # Consts on the scalar HWDGE ring (idle mid-run) so the sync
            # ring streams x immediately.
            w_sb = constp.tile([P, NT, C], bf16)
            nc.scalar.dma_start(
                out=w_sb[:].rearrange("p a b -> p (a b)"), in_=w_ext[:, :]
            )
            g_sb = constp.tile([P, 2], f32)
            nc.scalar.dma_start(out=g_sb[:], in_=g_ext[:])
            # onesA picks the [0:64) partition half (even chunks), onesB
            # the [64:128) half, both scaled 1/C for the channel mean.
            onesA = constp.tile([P, P], bf16)
            nc.vector.memset(onesA[0:C, :], 1.0 / C)
            nc.vector.memset(onesA[C:P, :], 0.0)
            onesB = constp.tile([P, P], bf16)
            nc.vector.memset(onesB[0:C, :], 0.0)
            nc.vector.memset(onesB[C:P, :], 1.0 / C)

            # One 4-bank PSUM tile for the mean waves (banks 4-7); cams
            # cycle banks 0-3. PE clock warm-up matmuls (never read; DCE
            # keeps unread matmuls) share the mean banks -- they finish
            # ~20us before the first mean matmul.
            mean_ps = psump.tile([P, NB, BANK], f32, name="mean_ps", tag="meanbank")
            w_flat = w_sb[:].rearrange("p a b -> p (a b)")
            for i in range(15):
                nc.tensor.matmul(
                    mean_ps[0:C, i % NB, :], w_sb[:, 0, :], w_flat[:, 0:BANK],
                    start=True, stop=True,
                )

            for n in range(NS):
                xbs = []
                for t in range(NT):
                    xb = xbp.tile([P, HW], bf16, name=f"xb_{n}_{t}", tag="xb")
                    nc.sync.dma_start(out=xb[:], in_=x_ext[n, t * P:(t + 1) * P, :])
                    xbs.append(xb)
                all_xbs.append(xbs)

                if n == NS - 1:
                    # Bulk-store gate: a dummy SWDGE transfer reading load
                    # tile 27 (~60us). The gpsimd engine is FIFO, so the
                    # deferred stores behind it cannot emit descriptors
                    # until then.
                    gate_sb = gatep.tile([1, 16], bf16, name="gate", tag="gate")
                    nc.gpsimd.dma_start(out=gate_sb[:], in_=xbs[3][0:1, 0:16])
                    for dn, dt in deferred:
                        nc.gpsimd.dma_start(
                            out=out_ext[dn, dt * P:(dt + 1) * P, :],
                            in_=all_xbs[dn][dt][:],
                        )

                # cams: chunk pair (2b, 2b+1) -> bank b partitions
                # [0:64)/[64:128) via PE tile_position (col base 64).
                cams = [
                    psump.tile([P, BANK], f32, name=f"cams_{n}_{b}", tag=f"bank{b}")
                    for b in range(NB)
                ]
                crelu = camp.tile([P, NB, CH], bf16, name=f"crelu_{n}", tag="crelu")

                def cam_mm(b, t):
                    nc.tensor.matmul(
                        cams[b][0:C, 0:CH],
                        w_sb[:, t, :],
                        xbs[t][:, (2 * b) * CH:(2 * b + 1) * CH],
                        start=(t == 0), stop=(t == NT - 1),
                    )
                    nc.tensor.matmul(
                        cams[b][C:P, 0:CH],
                        w_sb[:, t, :],
                        xbs[t][:, (2 * b + 1) * CH:(2 * b + 2) * CH],
                        start=(t == 0), stop=(t == NT - 1),
                    )

                if n == 0:
                    # t-outer (4 live banks, all free at startup) so the
                    # matmuls chase the initial tile loads.
                    for t in range(NT):
                        for b in range(NB):
                            cam_mm(b, t)
                    for b in range(NB):
                        nc.scalar.activation(
                            crelu[:, b, :], cams[b][:, 0:CH],
                            mybir.ActivationFunctionType.Relu,
                        )
                else:
                    # Bank-pair serial: 2 live banks per group; per-group
                    # relus evacuate banks a full group ahead of reuse.
                    for pair in ((0, 1), (2, 3)):
                        for t in range(NT):
                            for b in pair:
                                cam_mm(b, t)
                        for b in pair:
                            nc.scalar.activation(
                                crelu[:, b, :], cams[b][:, 0:CH],
                                mybir.ActivationFunctionType.Relu,
                            )

                # Spatial max on all 128 lanes; channel c's halves live on
                # lanes c and c+64, combined via two tiny cross-partition
                # SBUF->SBUF DMAs on the scalar ring (idle mid-run), then
                # thr = gama * max. max(crelu) == relu(max(cams)), so
                # comparing post-relu against thr >= 0 matches the
                # reference's pre-relu compare.
                cmax2 = statp.tile([P, 2], f32, name=f"cmax2_{n}", tag="cmax2")
                nc.vector.tensor_reduce(
                    cmax2[:, 0:1], crelu[:, 0:2, :], axis=mybir.AxisListType.XY,
                    op=alu.max,
                )
                nc.vector.tensor_reduce(
                    cmax2[:, 1:2], crelu[:, 2:4, :], axis=mybir.AxisListType.XY,
                    op=alu.max,
                )
                rmax = statp.tile([P, 1], f32, name=f"rmax_{n}", tag="rmax")
                nc.vector.tensor_reduce(
                    rmax[:], cmax2[:], axis=mybir.AxisListType.X, op=alu.max
                )
                rswap = statp.tile([P, 1], f32, name=f"rswap_{n}", tag="rswap")
                nc.scalar.dma_start(out=rswap[0:C, :], in_=rmax[C:P, :])
                nc.scalar.dma_start(out=rswap[C:P, :], in_=rmax[0:C, :])
                thr = statp.tile([P, 1], f32, name=f"thr_{n}", tag="thr")
                nc.vector.tensor_max(thr[:], rmax[:], rswap[:])
                nc.vector.tensor_scalar(
                    thr[:], thr[:], g_sb[:, 0:1], None, op0=alu.mult
                )

                # drop = crelu * (crelu <= thr), in place; two ops so the
                # first mean wave chases the first half.
                nc.vector.scalar_tensor_tensor(
                    crelu[:, 0:2, :], crelu[:, 0:2, :], thr[:],
                    crelu[:, 0:2, :], op0=alu.is_le, op1=alu.mult,
                )
                nc.vector.scalar_tensor_tensor(
                    crelu[:, 2:4, :], crelu[:, 2:4, :], thr[:],
                    crelu[:, 2:4, :], op0=alu.is_le, op1=alu.mult,
                )

                # Channel mean, broadcast to all 128 partitions: wave L
                # (chunks 0-3) then wave R (chunks 4-7), each 4 ones-
                # matmuls into the 4-bank mean tile + ONE strided ACT copy
                # out. mean chunk 2b+h = onesX(h) @ crelu[:, b, :].
                mean_sb = meanp.tile([P, HW], bf16, name=f"mean_{n}", tag="mean")
                for half in range(2):
                    for b2 in range(2):
                        b = half * 2 + b2
                        nc.tensor.matmul(
                            mean_ps[:, 2 * b2, 0:CH], onesA[:], crelu[:, b, :],
                            start=True, stop=True,
                        )
                        nc.tensor.matmul(
                            mean_ps[:, 2 * b2 + 1, 0:CH], onesB[:], crelu[:, b, :],
                            start=True, stop=True,
                        )
                    nc.scalar.copy(
                        mean_sb[:, half * (HW // 2):(half + 1) * (HW // 2)]
                        .rearrange("p (a b) -> p a b", a=NB),
                        mean_ps[:, :, 0:CH],
                    )

                # Products overwrite the xb tiles in place. Tile 0 in two
                # halves chasing the mean waves; all products stay on DVE
                # (GpSimd would contend for the shared SBUF port).
                for half in range(2):
                    sl = slice(half * (HW // 2), (half + 1) * (HW // 2))
                    nc.vector.tensor_mul(
                        xbs[0][:, sl], xbs[0][:, sl], mean_sb[:, sl]
                    )
                # Stores: last sample immediately on the scalar HWDGE ring;
                # sample 0 tiles 0-1 immediately on SWDGE (slots recycled
                # by loads 30-31); everything else deferred behind the gate.
                if n == NS - 1:
                    nc.scalar.dma_start(out=out_ext[n, 0:P, :], in_=xbs[0][:])
                elif n == 0:
                    nc.gpsimd.dma_start(out=out_ext[n, 0:P, :], in_=xbs[0][:])
                else:
                    deferred.append((n, 0))
                for t in range(1, NT):
                    nc.vector.tensor_mul(xbs[t][:], xbs[t][:], mean_sb[:])
                    if n == NS - 1:
                        nc.scalar.dma_start(
                            out=out_ext[n, t * P:(t + 1) * P, :], in_=xbs[t][:]
                        )
                    elif n == 0 and t == 1:
                        nc.gpsimd.dma_start(
                            out=out_ext[n, t * P:(t + 1) * P, :], in_=xbs[t][:]
                        )
                    else:
                        deferred.append((n, t))
    nc.compile()
    return nc


def _get_nc():
    if "nc" not in _CACHE:
        _CACHE["nc"] = _build_nc()
    return _CACHE["nc"]


def _make_in_maps(x, fc_weights, gama):
    from concourse import mybir

    bf16_np = mybir.dt.np(mybir.dt.bfloat16)
    x = np.asarray(x, dtype=np.float32)
    # [p, t*C+c] = w[c, t*128+p]: one contiguous [128, 512] block.
    w2 = np.ascontiguousarray(
        np.asarray(fc_weights, dtype=np.float32)
        .reshape(C, NT, P)
        .transpose(2, 1, 0)
        .reshape(P, NT * C)
    ).astype(bf16_np)
    g = np.asarray(gama, dtype=np.float32).reshape(1, 1)
    g128 = np.ascontiguousarray(
        np.broadcast_to(np.concatenate([g, -g], axis=1), (P, 2))
    )
    return [
        {
            "x": np.ascontiguousarray(
                x[i * NS:(i + 1) * NS].reshape(NS, CIN, HW)
            ).astype(bf16_np),
            "fc_weights": w2,
            "gama": g128,
        }
        for i in range(NCORES)
    ]


def kernel(x: np.ndarray, fc_weights: np.ndarray, gama: np.ndarray) -> np.ndarray:
    from concourse.bass_utils import run_bass_kernel_spmd

    nc = _get_nc()
    in_maps = _make_in_maps(x, fc_weights, gama)
    res = run_bass_kernel_spmd(nc, in_maps, core_ids=list(range(NCORES)))
    out = np.concatenate(
        [
            res.results[i]["out"].astype(np.float32).reshape(NS, CIN, H, W)
            for i in range(NCORES)
        ],
        axis=0,
    )
    return out


# revision 14
# speedup vs baseline: 1.1479x; 1.1479x over previous
"""Trainium2 Bass kernel for the CAM-drop attention module.

Reference computation (per sample n):
    cams  = relu(W @ x[n])            # W: [C=64, Cin=1024], x[n]: [Cin, H*W]
    thr_k = gama * max_hw(cams[k])    # per-channel spatial max
    drop  = where(cams > thr, 0, cams)
    out[n] = x[n] * mean_k(drop)      # broadcast over Cin

Data-parallel over the batch: 32 samples sharded 4-per-core across 8
NeuronCores; fc_weights / gama replicated. No cross-core communication.

The problem is HBM-bound, so x is pre-cast to bf16 on the host and loaded
as bf16, and the output is stored as bf16 and widened to f32 on the host
(halves both HBM streams; rel err stays ~7e-3, well under the 2e-2 gate).
Matmuls accumulate bf16 into f32 PSUM; the channel mean is bf16.

Per-core pipeline (samples unrolled):
  - x[n] streamed as 8 bf16 tiles [128, 3136] into a 30-slot rotating SBUF
    pool (3.75 samples of load prefetch); loads on the sync HWDGE ring,
    consts (w prelaid [128, 512] on host, gama) on the scalar ring so x
    bytes flow from ~8us (framework sem setup is the rest of the ramp)
  - cams accumulated in f32 PSUM with ONE TILE PER BANK (7 tags): PSUM
    dependency tracking at bank granularity, so relu_s chases the final
    matmul pass and sample n+1's matmuls chase the per-bank mean copies
  - per-bank relu (ACT) -> two partial spatial maxes + final (DVE),
    threshold, in-place drop-mask (DVE scalar_tensor_tensor)
  - channel mean via a bf16 [64->128] ones/64 matmul into the same per-bank
    PSUM slots, copied per-bank PSUM->SBUF on ACT
  - products IN PLACE: xb tile *= mean (DVE 2x tensor_tensor), stores read
    the xb tile; tile 0 chunked per bank to chase the copies with its store
    on the scalar HWDGE ring; tiles 1-7 stores on the gpsimd SWDGE ring,
    except the last sample's on the scalar ring (shorter completion drain)
  - host widens the bf16 output back to f32

Measured pitfalls baked into the structure: GpSimd tensor ops running
concurrently with DVE 2x-mode ops contend for SBUF ports and slow both
~4x; dense 4x-mode tensor_scalar activity trips HAM power throttling
(50%-duty windows); ScalarE ACTIVATE has no 16-bit accel (2.9us per
[64, 3136] op) so sign/compare paths stay off ACT.

Steady state is HBM-bus-bound at ~420 GB/s observed per core (51.4 MB
-> ~122us) with the DVE serial stream (~21.9us/sample) setting the
pipeline period and the tail.
"""

import numpy as np

# Problem shape (hardcoded per harness contract).
N, CIN, H, W = 32, 1024, 56, 56
C = 64
HW = H * W          # 3136
NCORES = 8
NS = N // NCORES    # 4 samples per core
P = 128             # SBUF partitions
NT = CIN // P       # 8 Cin tiles
NCH = 7             # spatial chunks per sample
CH = HW // NCH      # 448 (fits one PSUM bank)
BANK = 512          # PSUM bank stride in f32 elements
NBBUF = 30          # rotating bf16 x-tile slots (0.784 MB each)

_CACHE = {}


def _build_nc():
    from concourse import bacc, bass, tile
    from concourse import mybir

    f32 = mybir.dt.float32
    bf16 = mybir.dt.bfloat16
    alu = mybir.AluOpType

    nc = bacc.Bacc("TRN2", target_bir_lowering=False, debug=False)
    x_ext = nc.declare_dram_parameter("x", [NS, CIN, HW], bf16, isOutput=False)
    # fc_weights prelaid on host as [p, t*C+c] = w[c, t*128+p]: contiguous
    # 1KB partition lines -> one efficient DMA (the [CIN, C] layout's 128B
    # lines ran at ~24 GB/s and stalled the load ring for ~10us at startup).
    w_ext = nc.declare_dram_parameter("fc_weights", [P, NT * C], bf16, isOutput=False)
    g_ext = nc.declare_dram_parameter("gama", [C, 2], f32, isOutput=False)
    out_ext = nc.declare_dram_parameter("out", [NS, CIN, HW], bf16, isOutput=True)

    with tile.TileContext(nc) as tc:
        with (
            tc.tile_pool(name="consts", bufs=1) as constp,
            tc.tile_pool(name="xbp", bufs=NBBUF) as xbp,
            tc.tile_pool(name="stats", bufs=2) as statp,
            tc.tile_pool(name="camsb", bufs=1) as camp,
            tc.tile_pool(name="meanp", bufs=1) as meanp,
            tc.tile_pool(name="gatep", bufs=1) as gatep,
            tc.tile_pool(name="psum", bufs=1, space=bass.MemorySpace.PSUM) as psump,
        ):
            all_xbs = []
            deferred = []
            # Consts go on the scalar HWDGE ring so the sync ring starts
            # streaming x immediately (loads and consts in parallel).
            w_sb = constp.tile([P, NT, C], bf16)
            nc.scalar.dma_start(
                out=w_sb[:].rearrange("p a b -> p (a b)"), in_=w_ext[:, :]
            )
            # Columns: (gama, -gama).
            g_sb = constp.tile([C, 2], f32)
            nc.scalar.dma_start(out=g_sb[:], in_=g_ext[:])
            ones_sb = constp.tile([C, P], bf16)
            nc.vector.memset(ones_sb[:], 1.0 / C)

            # PE clock warm-up: the HAM gate holds the PE at half clock until
            # ~4us of sustained matmul activity. Garbage matmuls into a spare
            # PSUM bank (never read; DCE keeps unread matmuls) warm it up
            # during the initial load-only DMA phase.
            warm_ps = psump.tile([C, BANK], f32, name="warm_ps", tag="warm")
            w_flat = w_sb[:].rearrange("p a b -> p (a b)")
            for _ in range(15):
                nc.tensor.matmul(
                    warm_ps[:, :], w_sb[:, 0, :], w_flat[:, 0:BANK],
                    start=True, stop=True,
                )

            for n in range(NS):
                xbs = []
                for t in range(NT):
                    xb = xbp.tile([P, HW], bf16, name=f"xb_{n}_{t}", tag="xb")
                    nc.sync.dma_start(out=xb[:], in_=x_ext[n, t * P:(t + 1) * P, :])
                    xbs.append(xb)
                all_xbs.append(xbs)

                if n == NS - 1:
                    # Bulk-store gate: a dummy SWDGE transfer that reads a
                    # late load tile (tile 27, ~60us). The gpsimd engine is
                    # FIFO, so the deferred stores behind it cannot emit
                    # descriptors until then -- loads keep the full ~420
                    # GB/s to themselves until nearly done instead of
                    # splitting 50/50 with stores from ~37us (which pushed
                    # the last tile load, and the whole last-sample chain
                    # hanging off it, out to ~98us in the v4 trace).
                    gate_sb = gatep.tile([1, 16], bf16, name="gate", tag="gate")
                    nc.gpsimd.dma_start(out=gate_sb[:], in_=xbs[3][0:1, 0:16])
                    for dn, dt in deferred:
                        nc.gpsimd.dma_start(
                            out=out_ext[dn, dt * P:(dt + 1) * P, :],
                            in_=all_xbs[dn][dt][:],
                        )

                # PSUM budget: cams(n>=1) cycle banks 0-3 only (chunk-pair
                # serial below), mean uses banks 4-6, so sample n+1's cam
                # matmuls never wait on sample n's mean copies -- the v3
                # trace showed that bank coupling as 10-14us DVE idle gaps
                # before samples 2 and 3.
                cams = [
                    psump.tile([P, BANK], f32, name=f"cams_{n}_{s}",
                               tag=f"bank{s if n == 0 else s % 4}")
                    for s in range(NCH)
                ]
                crelu = camp.tile([C, NCH, CH], bf16, name=f"crelu_{n}", tag="crelu")
                if n == 0:
                    # Sample 0: t-outer (7 live banks) so matmuls chase the
                    # initial tile loads; banks are all free at startup.
                    for t in range(NT):
                        for s in range(NCH):
                            nc.tensor.matmul(
                                cams[s][0:C, 0:CH],
                                w_sb[:, t, :],
                                xbs[t][:, s * CH:(s + 1) * CH],
                                start=(t == 0),
                                stop=(t == NT - 1),
                            )
                    for s in range(NCH):
                        nc.scalar.activation(
                            crelu[:, s, :], cams[s][0:C, 0:CH],
                            mybir.ActivationFunctionType.Relu,
                        )
                else:
                    # Samples 1+: chunk-pair serial -- only 2 banks live per
                    # group, cycling banks 0-3; per-group relus evacuate
                    # banks two groups ahead of reuse. Loads are prefetched
                    # ~3.5 samples ahead so the t-inner order never stalls.
                    for chunks in ((0, 1), (2, 3), (4, 5), (6,)):
                        for t in range(NT):
                            for s in chunks:
                                nc.tensor.matmul(
                                    cams[s][0:C, 0:CH],
                                    w_sb[:, t, :],
                                    xbs[t][:, s * CH:(s + 1) * CH],
                                    start=(t == 0),
                                    stop=(t == NT - 1),
                                )
                        for s in chunks:
                            nc.scalar.activation(
                                crelu[:, s, :], cams[s][0:C, 0:CH],
                                mybir.ActivationFunctionType.Relu,
                            )
                # Spatial max in two partials chasing the relus; final max
                # combines. max(crelu) == relu(max(cams)), so thr =
                # max(crelu) * gama directly (and -thr via the -gama col).
                cmax2 = statp.tile([C, 2], f32, name=f"cmax2_{n}", tag="cmax2")
                nc.vector.tensor_reduce(
                    cmax2[:, 0:1], crelu[:, 0:4, :], axis=mybir.AxisListType.XY,
                    op=alu.max,
                )
                nc.vector.tensor_reduce(
                    cmax2[:, 1:2], crelu[:, 4:NCH, :], axis=mybir.AxisListType.XY,
                    op=alu.max,
                )
                cmax = statp.tile([C, 1], f32, name=f"cmax_{n}", tag="cmax")
                nc.vector.tensor_reduce(
                    cmax[:], cmax2[:], axis=mybir.AxisListType.X, op=alu.max
                )
                thr = statp.tile([C, 1], f32, name=f"thr_{n}", tag="thr")
                nc.vector.tensor_scalar(
                    thr[:], cmax[:], g_sb[:, 0:1], None, op0=alu.mult
                )

                # drop = crelu * (crelu <= thr), in place (comparing post-relu
                # values against thr >= 0 matches the reference's pre-relu
                # compare). Then the channel mean, broadcast to all 128
                # partitions via a ones/64 matmul into banks 4-6 (disjoint
                # from the cams cycle on banks 0-3).
                mean_ps = [
                    psump.tile([P, BANK], f32, name=f"meanps_{n}_{s}",
                               tag=f"bank{4 + s % 3}")
                    for s in range(NCH)
                ]
                mean_sb = meanp.tile([P, HW], bf16, name=f"mean_{n}", tag="mean")
                mean_sb3 = mean_sb[:].rearrange("p (a b) -> p a b", a=NCH)
                # Mask as the fused scalar_tensor_tensor (1x mode but a
                # single pass): splitting it into a 4x is_le + 2x multiply
                # measured WORSE end-to-end -- the denser 4x op activity
                # trips HAM power throttling (50% duty windows).
                for s0, s1 in ((0, 4), (4, NCH)):
                    nc.vector.scalar_tensor_tensor(
                        crelu[:, s0:s1, :], crelu[:, s0:s1, :], thr[:],
                        crelu[:, s0:s1, :], op0=alu.is_le, op1=alu.mult,
                    )
                for s in range(NCH):
                    nc.tensor.matmul(
                        mean_ps[s][:, 0:CH], ones_sb[:], crelu[:, s, :],
                        start=True, stop=True,
                    )
                # Mean copies on DVE (tensor_copy, 1x PSUM mode, ~0.6us
                # each), NOT ACT: Tile lowers cross-engine deps to
                # completion COUNTERS, so if the scheduler orders ACT as
                # [relus(n+1) ... copies(n)], products(n) transitively wait
                # on sample n+1's LOADS (measured as a 9us DVE stall before
                # products(2) in the v6 trace). On DVE the mask -> mean ->
                # product chain is self-paced on one engine.
                for s in range(NCH):
                    nc.vector.tensor_copy(mean_sb3[:, s, :], mean_ps[s][:, 0:CH])

                # Products overwrite the xb tiles in place (no separate out
                # pool -> 6 more xb slots of load prefetch). Tile 0 right
                # after the copies on the same engine; all products stay on
                # DVE: a GpSimd tensor op running concurrently with DVE
                # 2x-mode ops contends for SBUF ports and slows BOTH ~4x
                # (measured 1.78us -> 7.7us).
                nc.vector.tensor_mul(xbs[0][:], xbs[0][:], mean_sb[:])
                # Stores: the last sample's go out immediately on the scalar
                # HWDGE ring (loads are done by then; HWDGE completion is
                # ~0.6us vs ~2us SWDGE, shortening the final drain). Sample
                # 0's tiles 0-1 also store immediately (their SBUF slots are
                # recycled by loads 30-31, which must not wait for the
                # gate). Everything else is deferred behind the gate above.
                if n == NS - 1:
                    nc.scalar.dma_start(out=out_ext[n, 0:P, :], in_=xbs[0][:])
                elif n == 0:
                    nc.gpsimd.dma_start(out=out_ext[n, 0:P, :], in_=xbs[0][:])
                else:
                    deferred.append((n, 0))
                for t in range(1, NT):
                    nc.vector.tensor_mul(xbs[t][:], xbs[t][:], mean_sb[:])
                    if n == NS - 1:
                        nc.scalar.dma_start(
                            out=out_ext[n, t * P:(t + 1) * P, :], in_=xbs[t][:]
                        )
                    elif n == 0 and t == 1:
                        nc.gpsimd.dma_start(
                            out=out_ext[n, t * P:(t + 1) * P, :], in_=xbs[t][:]
                        )
                    else:
                        deferred.append((n, t))
    nc.compile()
    return nc


def _get_nc():
    if "nc" not in _CACHE:
        _CACHE["nc"] = _build_nc()
    return _CACHE["nc"]


def _make_in_maps(x, fc_weights, gama):
    from concourse import mybir

    bf16_np = mybir.dt.np(mybir.dt.bfloat16)
    x = np.asarray(x, dtype=np.float32)
    # [p, t*C+c] = w[c, t*128+p]: one contiguous [128, 512] block.
    w2 = np.ascontiguousarray(
        np.asarray(fc_weights, dtype=np.float32)
        .reshape(C, NT, P)
        .transpose(2, 1, 0)
        .reshape(P, NT * C)
    ).astype(bf16_np)
    g = np.asarray(gama, dtype=np.float32).reshape(1, 1)
    g64 = np.ascontiguousarray(
        np.broadcast_to(np.concatenate([g, -g], axis=1), (C, 2))
    )
    return [
        {
            "x": np.ascontiguousarray(
                x[i * NS:(i + 1) * NS].reshape(NS, CIN, HW)
            ).astype(bf16_np),
            "fc_weights": w2,
            "gama": g64,
        }
        for i in range(NCORES)
    ]


def kernel(x: np.ndarray, fc_weights: np.ndarray, gama: np.ndarray) -> np.ndarray:
    from concourse.bass_utils import run_bass_kernel_spmd

    nc = _get_nc()
    in_maps = _make_in_maps(x, fc_weights, gama)
    res = run_bass_kernel_spmd(nc, in_maps, core_ids=list(range(NCORES)))
    out = np.concatenate(
        [
            res.results[i]["out"].astype(np.float32).reshape(NS, CIN, H, W)
            for i in range(NCORES)
        ],
        axis=0,
    )
    return out

